# revision 16
# baseline (speedup 1.0000x reference)
"""Associative-embedding (push/pull) loss on 8 TRN2 NeuronCores.

Data-parallel: 8 images per core. The 285MB tags tensor is only touched at
P*K=510 points per image, so each core row-gathers 64-element windows with
dma_gather (one CounterMachine-accelerated SWDGE instruction per image,
640 int16 row indices each) instead of streaming the tensor:

  element e = 65536*(img*17+k) + 256*x + y
  row r (64-wide) = k*1024 + 4*x + (y>>6), selected within-window by y&63.

Gathered rows land as out[g%128, g//128, :] with g = 128*(k//4) + 32*(k%4)
+ p, i.e. partition q = 32*(k%4)+p -> a 120-partition-parallel layout for
the one-hot within-window select and the masked moment sums. A 128->30
fold matmul brings per-(person,img) moments back to person partitions;
pairwise push runs in an [img, person] layout after a 32x32 DVE block
transpose. Per-core (sum_push, sum_pull)/64 partials are summed across
cores (host side, or on-device AllReduce with AELOSS_COLLECTIVE=1).
"""

import os
import sys

import numpy as np

if "/opt/trn_rl_repo" not in sys.path:
    sys.path.insert(0, "/opt/trn_rl_repo")

from concourse import bacc, bass, mybir, tile  # noqa: E402
from concourse import bass_utils  # noqa: E402
from concourse.masks import make_identity  # noqa: E402

B, P, K, H, W = 64, 30, 17, 256, 256
NCORES = 8
BPC = B // NCORES           # 8 images per core
J = BPC * K                 # 136 (img, k) columns
KHW = K * H * W             # 1114112
NTOT = BPC * KHW
NROW = KHW // 64            # 17408 64-elem rows per image
NIDX = 544                  # 17 k * 32 partitions per image (g = 32k + p)
NS = 5                      # k slots per image (slot = k // 4)

f32 = mybir.dt.float32
i32 = mybir.dt.int32
i16 = mybir.dt.int16
Alu = mybir.AluOpType
Act = mybir.ActivationFunctionType
AX = mybir.AxisListType


def build_nc(collective: bool = False):
    nc = bacc.Bacc("TRN2", target_bir_lowering=False, debug=False,
                   num_devices=NCORES)

    tags = nc.dram_tensor("tags", [BPC, NROW, 64], f32, kind="ExternalInput")
    joints = nc.dram_tensor("joints", [BPC, P, K, 2], i32, kind="ExternalInput")
    jv = nc.dram_tensor("jv", [BPC, P, K], i32, kind="ExternalInput")
    pv = nc.dram_tensor("pv", [BPC, P], i32, kind="ExternalInput")
    out = nc.dram_tensor("out", [2, 1], f32, kind="ExternalOutput")

    with tile.TileContext(nc) as tc:
        with tc.tile_pool(name="sbuf", bufs=1) as pool, \
             tc.tile_pool(name="psum", bufs=1, space="PSUM") as psp:

            # ---- load the small tensors ----
            # joints twice: [p, (img,k), xy] for the ylow/select path, and
            # folded [p%16, p//16, img, k, xy] to feed the gather-index build
            # without needing person-fold matmuls.
            jnt = pool.tile([P, J, 2], i32)
            jnt16 = pool.tile([16, 2, BPC, K, 2], i32)
            jvt = pool.tile([P, J], i32)
            pvt = pool.tile([P, BPC], i32)
            nc.sync.dma_start(out=jnt[:],
                              in_=joints[:].rearrange("b p k c -> p b k c"))
            nc.vector.memset(jnt16[:], 0)
            nc.sync.dma_start(
                out=jnt16[:, 0],
                in_=joints[:, 0:16].rearrange("b r k c -> r b k c"))
            nc.sync.dma_start(
                out=jnt16[0:14, 1],
                in_=joints[:, 16:30].rearrange("b r k c -> r b k c"))
            nc.sync.dma_start(out=jvt[:], in_=jv[:].rearrange("b p k -> p b k"))
            nc.sync.dma_start(out=pvt[:], in_=pv[:].rearrange("b p -> p b"))

            # ---- constants (iotas, selection matrices) ----
            # replicate matrix [16, 128]: rep[r, q] = (q % 16 == r)
            iot_f16 = pool.tile([16, 128], i32)
            iot_c16 = pool.tile([16, 128], i32)
            rep = pool.tile([16, 128], f32)
            nc.gpsimd.iota(iot_f16[:], pattern=[[0, 8], [1, 16]], base=0,
                           channel_multiplier=0)
            nc.gpsimd.iota(iot_c16[:], pattern=[[0, 128]], base=0,
                           channel_multiplier=1)
            nc.vector.tensor_tensor(out=rep[:], in0=iot_f16[:], in1=iot_c16[:],
                                    op=Alu.is_equal)
            # fold matrix [128, 30]: foldp[q, p] = (q % 32 == p)
            iot_q = pool.tile([128, 1], i32)
            iot_r30 = pool.tile([128, 30], i32)
            foldp = pool.tile([128, 30], f32)
            nc.gpsimd.iota(iot_q[:], pattern=[[0, 1]], base=0,
                           channel_multiplier=1)
            nc.vector.tensor_scalar(out=iot_q[:], in0=iot_q[:], scalar1=31,
                                    scalar2=None, op0=Alu.bitwise_and)
            nc.gpsimd.iota(iot_r30[:], pattern=[[1, 30]], base=0,
                           channel_multiplier=0)
            nc.vector.tensor_tensor(out=foldp[:], in0=iot_r30[:],
                                    in1=iot_q[:].to_broadcast([128, 30]),
                                    op=Alu.is_equal)
            # within-window iota [128, NS, 64] (value = w)
            iot_w = pool.tile([128, NS, 64], i32)
            nc.gpsimd.iota(iot_w[:], pattern=[[0, NS], [1, 64]], base=0,
                           channel_multiplier=0)

            # ---- ylow = y & 63 in the [p, (img,k)] layout ----
            ylow = pool.tile([P, J], i32)
            nc.vector.tensor_scalar(out=ylow[:], in0=jnt[:, :, 1], scalar1=63,
                                    scalar2=None, op0=Alu.bitwise_and)

            # ---- gather row index on the folded layout ----
            # rY = 1024*k + 4*x + (y>>6) computed at [p%16, (p//16, img, k)]
            kb16 = pool.tile([16, 2, BPC, K], i32)
            nc.gpsimd.iota(kb16[:], pattern=[[0, 2], [0, BPC], [1024, K]],
                           base=0, channel_multiplier=0)
            rY16 = pool.tile([16, 2, BPC, K], i32)
            nc.vector.tensor_scalar(out=rY16[:], in0=jnt16[:, :, :, :, 1],
                                    scalar1=6, scalar2=None,
                                    op0=Alu.arith_shift_right)
            nc.vector.tensor_tensor(out=rY16[:], in0=rY16[:], in1=kb16[:],
                                    op=Alu.add)
            nc.vector.scalar_tensor_tensor(out=rY16[:],
                                           in0=jnt16[:, :, :, :, 0], scalar=4,
                                           in1=rY16[:], op0=Alu.mult,
                                           op1=Alu.add)
            rYf16 = pool.tile([16, 2 * J], f32)
            nc.vector.tensor_copy(out=rYf16[:],
                                  in_=rY16[:].rearrange("r j b k -> r (j b k)"))

            # replicate [16] -> [128] partitions so each gpsimd core sees the
            # wrapped indices, then one cast into the int16 index tile.
            # g = 32*k + p: idx16 slot [p%16, 2*k + p//16].
            psR = psp.tile([128, 2 * J], f32)
            nc.tensor.matmul(out=psR[:], lhsT=rep[:], rhs=rYf16[:],
                             start=True, stop=True)
            idx16 = pool.tile([128, BPC, NIDX // 16], i16)
            nc.vector.tensor_copy(
                out=idx16[:].rearrange("q b (k j) -> q b j k", j=2),
                in_=psR[:].rearrange("q (j b k) -> q b j k", j=2, b=BPC))

            # ---- the 8 row gathers ----
            rows = pool.tile([128, BPC, NS, 64], f32)
            nc.vector.memset(rows[:, :, NS - 1, :], 0.0)
            for i in range(BPC):
                nc.gpsimd.dma_gather(
                    out_ap=rows[:, i, :, :],
                    in_ap=tags[i],
                    idxs_ap=idx16[:, i, :],
                    num_idxs=NIDX,
                    num_idxs_reg=NIDX,
                    elem_size=64,
                )

            # ---- scramble ylow / jv-mask into the [32a+p, (i,s)] layout ----
            mjv = pool.tile([P, J], f32)
            nc.vector.tensor_scalar(out=mjv[:], in0=jvt[:], scalar1=0,
                                    scalar2=None, op0=Alu.is_gt)
            ylow_s = pool.tile([128, BPC, NS], i32)
            jvm_s = pool.tile([128, BPC, NS], f32)
            nc.vector.memset(ylow_s[:], 0)
            nc.vector.memset(jvm_s[:], 0.0)
            ylow4 = ylow[:].rearrange("p (b k) -> p b k", k=K)[:, :, 0:16] \
                .rearrange("p b (s a) -> p b a s", a=4)
            mjv4 = mjv[:].rearrange("p (b k) -> p b k", k=K)[:, :, 0:16] \
                .rearrange("p b (s a) -> p b a s", a=4)
            for a in range(4):
                nc.vector.tensor_copy(out=ylow_s[a * 32:a * 32 + P, :, 0:4],
                                      in_=ylow4[:, :, a, :])
                nc.vector.tensor_copy(out=jvm_s[a * 32:a * 32 + P, :, 0:4],
                                      in_=mjv4[:, :, a, :])
            # k = 16 lives in block a=0, slot s=4
            nc.vector.tensor_copy(
                out=ylow_s[0:P, :, 4],
                in_=ylow[:].rearrange("p (b k) -> p b k", k=K)[:, :, 16])
            nc.vector.tensor_copy(
                out=jvm_s[0:P, :, 4],
                in_=mjv[:].rearrange("p (b k) -> p b k", k=K)[:, :, 16])

            # ---- within-window select + masked moments, per image ----
            v_s = pool.tile([128, BPC, NS], f32)
            eq = pool.tile([128, NS, 64], f32)
            prod = pool.tile([128, NS, 64], f32)
            for i in range(BPC):
                nc.vector.tensor_tensor(
                    out=eq[:], in0=iot_w[:],
                    in1=ylow_s[:, i, :].unsqueeze(2).to_broadcast(
                        [128, NS, 64]),
                    op=Alu.is_equal)
                nc.vector.tensor_tensor(out=prod[:], in0=eq[:],
                                        in1=rows[:, i, :, :], op=Alu.mult)
                nc.vector.tensor_reduce(out=v_s[:, i, :], in_=prod[:],
                                        axis=AX.X, op=Alu.add)

            mv = pool.tile([128, BPC, NS], f32)
            mv2 = pool.tile([128, BPC, NS], f32)
            nc.vector.tensor_tensor(out=mv[:], in0=jvm_s[:], in1=v_s[:],
                                    op=Alu.mult)
            nc.vector.tensor_tensor(out=mv2[:], in0=mv[:], in1=v_s[:],
                                    op=Alu.mult)
            stack = pool.tile([128, 3, BPC], f32)
            nc.vector.tensor_reduce(out=stack[:, 0, :], in_=jvm_s[:],
                                    axis=AX.X, op=Alu.add)
            nc.vector.tensor_reduce(out=stack[:, 1, :], in_=mv[:],
                                    axis=AX.X, op=Alu.add)
            nc.vector.tensor_reduce(out=stack[:, 2, :], in_=mv2[:],
                                    axis=AX.X, op=Alu.add)

            # fold 128 -> 30 partitions and apply the person-valid mask
            psM = psp.tile([30, 3, BPC], f32)
            nc.tensor.matmul(out=psM[:].rearrange("p t b -> p (t b)"),
                             lhsT=foldp[:],
                             rhs=stack[:].rearrange("q t b -> q (t b)"),
                             start=True, stop=True)
            mpv = pool.tile([P, BPC], f32)
            nc.vector.tensor_scalar(out=mpv[:], in0=pvt[:], scalar1=0,
                                    scalar2=None, op0=Alu.is_gt)
            mom = pool.tile([P, 3, BPC], f32)
            nc.vector.tensor_tensor(
                out=mom[:], in0=psM[:],
                in1=mpv[:].unsqueeze(1).to_broadcast([P, 3, BPC]),
                op=Alu.mult)
            cnt = mom[:, 0, :]
            s1 = mom[:, 1, :]
            s2 = mom[:, 2, :]

            safe = pool.tile([P, BPC], f32)
            inv = pool.tile([P, BPC], f32)
            nc.vector.tensor_scalar(out=safe[:], in0=cnt, scalar1=1.0,
                                    scalar2=None, op0=Alu.max)
            nc.vector.reciprocal(out=inv[:], in_=safe[:])

            # staging tile for the 32x32 block transpose:
            # block0 = mean, block1 = pm, block2 = pull_p
            t_in = pool.tile([32, 96], f32)
            t_out = pool.tile([32, 96], f32)
            nc.vector.memset(t_in[:], 0.0)
            mean = t_in[0:P, 0:BPC]
            pm = t_in[0:P, 32:32 + BPC]
            pullp = t_in[0:P, 64:64 + BPC]
            nc.vector.tensor_tensor(out=mean, in0=s1, in1=inv[:], op=Alu.mult)
            nc.vector.tensor_scalar(out=pm, in0=cnt, scalar1=0.0,
                                    scalar2=None, op0=Alu.is_gt)
            tmp = pool.tile([P, BPC], f32)
            nc.vector.tensor_tensor(out=tmp[:], in0=s1, in1=mean, op=Alu.mult)
            nc.vector.tensor_tensor(out=tmp[:], in0=s2, in1=tmp[:],
                                    op=Alu.subtract)
            nc.vector.tensor_tensor(out=pullp, in0=tmp[:], in1=inv[:],
                                    op=Alu.mult)

            nc.vector.transpose(out=t_out[:], in_=t_in[:])
            meanT = t_out[0:BPC, 0:P]          # [8, 30]
            pmT = t_out[0:BPC, 32:32 + P]
            pullT = t_out[0:BPC, 64:64 + P]

            # ---- push: pairwise exp(-(mean_p - mean_q)^2) ----
            d2 = pool.tile([BPC, P, P], f32)
            nc.vector.tensor_tensor(
                out=d2[:],
                in0=meanT.unsqueeze(2).to_broadcast([BPC, P, P]),
                in1=meanT.unsqueeze(1).to_broadcast([BPC, P, P]),
                op=Alu.subtract)
            nc.vector.tensor_tensor(out=d2[:], in0=d2[:], in1=d2[:],
                                    op=Alu.mult)
            e = pool.tile([BPC, P, P], f32)
            nc.scalar.activation(out=e[:], in_=d2[:], func=Act.Exp, scale=-1.0)
            pair = pool.tile([BPC, P, P], f32)
            nc.vector.tensor_tensor(
                out=pair[:],
                in0=pmT.unsqueeze(2).to_broadcast([BPC, P, P]),
                in1=pmT.unsqueeze(1).to_broadcast([BPC, P, P]),
                op=Alu.mult)
            ep = pool.tile([BPC, P, P], f32)
            s_img = pool.tile([BPC, 1], f32)
            nc.vector.tensor_tensor(out=ep[:], in0=e[:], in1=pair[:],
                                    op=Alu.mult)
            nc.vector.tensor_reduce(out=s_img[:], in_=ep[:], axis=AX.XY,
                                    op=Alu.add)

            n_img = pool.tile([BPC, 1], f32)
            pull_sum = pool.tile([BPC, 1], f32)
            nc.vector.tensor_reduce(out=n_img[:], in_=pmT, axis=AX.X,
                                    op=Alu.add)
            nc.vector.tensor_reduce(out=pull_sum[:], in_=pullT, axis=AX.X,
                                    op=Alu.add)

            # push_img = (s - n) * 0.5 / max(n^2-n, 1) * (n > 1)
            pp2 = pool.tile([BPC, 2], f32)
            den = pool.tile([BPC, 1], f32)
            dinv = pool.tile([BPC, 1], f32)
            g = pool.tile([BPC, 1], f32)
            u = pool.tile([BPC, 1], f32)
            nc.vector.tensor_tensor(out=den[:], in0=n_img[:], in1=n_img[:],
                                    op=Alu.mult)
            nc.vector.tensor_tensor(out=den[:], in0=den[:], in1=n_img[:],
                                    op=Alu.subtract)
            nc.vector.tensor_scalar(out=den[:], in0=den[:], scalar1=1.0,
                                    scalar2=None, op0=Alu.max)
            nc.vector.reciprocal(out=dinv[:], in_=den[:])
            nc.vector.tensor_scalar(out=g[:], in0=n_img[:], scalar1=1.0,
                                    scalar2=None, op0=Alu.is_gt)
            nc.vector.tensor_tensor(out=u[:], in0=s_img[:], in1=n_img[:],
                                    op=Alu.subtract)
            nc.vector.tensor_tensor(out=u[:], in0=u[:], in1=dinv[:],
                                    op=Alu.mult)
            nc.vector.tensor_tensor(out=u[:], in0=u[:], in1=g[:], op=Alu.mult)
            nc.vector.tensor_scalar(out=pp2[:, 0:1], in0=u[:], scalar1=0.5,
                                    scalar2=None, op0=Alu.mult)

            # pull_img = pull_sum / max(n, 1)
            nm = pool.tile([BPC, 1], f32)
            ninv = pool.tile([BPC, 1], f32)
            nc.vector.tensor_scalar(out=nm[:], in0=n_img[:], scalar1=1.0,
                                    scalar2=None, op0=Alu.max)
            nc.vector.reciprocal(out=ninv[:], in_=nm[:])
            nc.vector.tensor_tensor(out=pp2[:, 1:2], in0=pull_sum[:],
                                    in1=ninv[:], op=Alu.mult)

            # ---- sum over the 8 local images: psum[2,1] = pp2^T @ ones ----
            ones8 = pool.tile([BPC, 1], f32)
            nc.vector.memset(ones8[:], 1.0)
            acc = psp.tile([2, 1], f32)
            nc.tensor.matmul(out=acc[:], lhsT=pp2[:], rhs=ones8[:],
                             start=True, stop=True)
            res = pool.tile([2, 1], f32)
            nc.scalar.activation(out=res[:], in_=acc[:], func=Act.Copy,
                                 scale=1.0 / B)

            if collective:
                with tc.tile_pool(name="dram", bufs=1, space="DRAM") as dram:
                    ar_in = dram.tile([2, 1], f32)
                    ar_out = dram.tile([2, 1], f32)
                    nc.sync.dma_start(out=ar_in[:], in_=res[:])
                    nc.gpsimd.collective_compute(
                        "AllReduce", Alu.add,
                        replica_groups=[list(range(NCORES))],
                        ins=[ar_in.opt()],
                        outs=[ar_out.opt()],
                    )
                    nc.sync.dma_start(out=out[:], in_=ar_out[:])
            else:
                nc.sync.dma_start(out=out[:], in_=res[:])

    nc.compile()
    return nc


_nc_cache = {}


def _get_nc(collective: bool):
    if collective not in _nc_cache:
        _nc_cache[collective] = build_nc(collective)
    return _nc_cache[collective]


def _shard_tags(shard):
    return np.ascontiguousarray(shard.reshape(BPC, NROW, 64))


def kernel(tags, joints, joint_img_valid, person_valid):
    collective = os.environ.get("AELOSS_COLLECTIVE", "0") == "1"
    nc = _get_nc(collective)

    tags = np.asarray(tags, dtype=np.float32)
    joints = np.ascontiguousarray(np.asarray(joints, dtype=np.int32))
    jv = np.ascontiguousarray(np.asarray(joint_img_valid, dtype=np.int32))
    pv = np.ascontiguousarray(np.asarray(person_valid, dtype=np.int32))

    in_maps = []
    for c in range(NCORES):
        sl = slice(c * BPC, (c + 1) * BPC)
        in_maps.append({
            "tags": _shard_tags(tags[sl]),
            "joints": joints[sl],
            "jv": jv[sl],
            "pv": pv[sl],
        })

    res = bass_utils.run_bass_kernel_spmd(nc, in_maps,
                                          core_ids=list(range(NCORES)))
    outs = [np.asarray(r["out"], dtype=np.float64).reshape(2)
            for r in res.results]
    if collective:
        total = outs[0]
    else:
        total = np.sum(outs, axis=0)
    return np.float32(total[0]), np.float32(total[1])


if __name__ == "__main__":
    rng = np.random.default_rng(0)
    t = rng.standard_normal((B, K, H, W), dtype=np.float32)
    j = rng.integers(0, H, size=(B, P, K, 2), dtype=np.int32)
    jv_ = rng.integers(0, 2, size=(B, P, K), dtype=np.int32)
    pv_ = rng.integers(0, 2, size=(B, P), dtype=np.int32)
    print(kernel(t, j, jv_, pv_))


# revision 17
# speedup vs baseline: 1.1928x; 1.1928x over previous
"""Associative-embedding (push/pull) loss on 8 TRN2 NeuronCores.

Data-parallel: 8 images per core. The 285MB tags tensor is only touched at
P*K=510 points per image, so each core row-gathers 64-element windows with
dma_gather (one CounterMachine-accelerated SWDGE instruction per image,
640 int16 row indices each) instead of streaming the tensor:

  element e = 65536*(img*17+k) + 256*x + y
  row r (64-wide) = k*1024 + 4*x + (y>>6), selected within-window by y&63.

Gathered rows land as out[g%128, g//128, :] with g = 128*(k//4) + 32*(k%4)
+ p, i.e. partition q = 32*(k%4)+p -> a 120-partition-parallel layout for
the one-hot within-window select and the masked moment sums. A 128->30
fold matmul brings per-(person,img) moments back to person partitions;
pairwise push runs in an [img, person] layout after a 32x32 DVE block
transpose. Per-core (sum_push, sum_pull)/64 partials are summed across
cores (host side, or on-device AllReduce with AELOSS_COLLECTIVE=1).
"""

import os
import sys

import numpy as np

if "/opt/trn_rl_repo" not in sys.path:
    sys.path.insert(0, "/opt/trn_rl_repo")

from concourse import bacc, bass, mybir, tile  # noqa: E402
from concourse import bass_utils  # noqa: E402
from concourse.masks import make_identity  # noqa: E402

B, P, K, H, W = 64, 30, 17, 256, 256
NCORES = 8
BPC = B // NCORES           # 8 images per core
J = BPC * K                 # 136 (img, k) columns
KHW = K * H * W             # 1114112
NTOT = BPC * KHW
NROW = KHW // 64            # 17408 64-elem rows per image
NIDX = 544                  # 17 k * 32 partitions per image (g = 32k + p)
NS = 5                      # k slots per image (slot = k // 4)

f32 = mybir.dt.float32
i32 = mybir.dt.int32
i16 = mybir.dt.int16
Alu = mybir.AluOpType
Act = mybir.ActivationFunctionType
AX = mybir.AxisListType


def build_nc(collective: bool = False):
    nc = bacc.Bacc("TRN2", target_bir_lowering=False, debug=False,
                   num_devices=NCORES)

    tags = nc.dram_tensor("tags", [BPC, NROW, 64], f32, kind="ExternalInput")
    joints = nc.dram_tensor("joints", [BPC, P, K, 2], i32, kind="ExternalInput")
    jv = nc.dram_tensor("jv", [BPC, P, K], i32, kind="ExternalInput")
    pv = nc.dram_tensor("pv", [BPC, P], i32, kind="ExternalInput")
    out = nc.dram_tensor("out", [2, 1], f32, kind="ExternalOutput")

    with tile.TileContext(nc) as tc:
        with tc.tile_pool(name="sbuf", bufs=1) as pool, \
             tc.tile_pool(name="psum", bufs=1, space="PSUM") as psp:

            # ---- load the small tensors ----
            # joints twice: [p, (img,k), xy] for the ylow/select path, and
            # folded [p%16, p//16, img, k, xy] to feed the gather-index build
            # without needing person-fold matmuls.
            jnt = pool.tile([P, J, 2], i32)
            jnt16 = pool.tile([16, 2, BPC, K, 2], i32)
            jvt = pool.tile([P, J], i32)
            pvt = pool.tile([P, BPC], i32)
            nc.sync.dma_start(out=jnt[:],
                              in_=joints[:].rearrange("b p k c -> p b k c"))
            nc.vector.memset(jnt16[:], 0)
            nc.sync.dma_start(
                out=jnt16[:, 0],
                in_=joints[:, 0:16].rearrange("b r k c -> r b k c"))
            nc.sync.dma_start(
                out=jnt16[0:14, 1],
                in_=joints[:, 16:30].rearrange("b r k c -> r b k c"))
            nc.sync.dma_start(out=jvt[:], in_=jv[:].rearrange("b p k -> p b k"))
            nc.sync.dma_start(out=pvt[:], in_=pv[:].rearrange("b p -> p b"))

            # ---- constants (iotas, selection matrices) ----
            # replicate matrix [16, 128]: rep[r, q] = (q % 16 == r)
            iot_f16 = pool.tile([16, 128], i32)
            iot_c16 = pool.tile([16, 128], i32)
            rep = pool.tile([16, 128], f32)
            nc.gpsimd.iota(iot_f16[:], pattern=[[0, 8], [1, 16]], base=0,
                           channel_multiplier=0)
            nc.gpsimd.iota(iot_c16[:], pattern=[[0, 128]], base=0,
                           channel_multiplier=1)
            nc.vector.tensor_tensor(out=rep[:], in0=iot_f16[:], in1=iot_c16[:],
                                    op=Alu.is_equal)
            # fold matrix [128, 30]: foldp[q, p] = (q % 32 == p)
            iot_q = pool.tile([128, 1], i32)
            iot_r30 = pool.tile([128, 30], i32)
            foldp = pool.tile([128, 30], f32)
            nc.gpsimd.iota(iot_q[:], pattern=[[0, 1]], base=0,
                           channel_multiplier=1)
            nc.vector.tensor_scalar(out=iot_q[:], in0=iot_q[:], scalar1=31,
                                    scalar2=None, op0=Alu.bitwise_and)
            nc.gpsimd.iota(iot_r30[:], pattern=[[1, 30]], base=0,
                           channel_multiplier=0)
            nc.vector.tensor_tensor(out=foldp[:], in0=iot_r30[:],
                                    in1=iot_q[:].to_broadcast([128, 30]),
                                    op=Alu.is_equal)
            # within-window iota [128, NS, 64] (value = w)
            iot_w = pool.tile([128, NS, 64], i32)
            nc.gpsimd.iota(iot_w[:], pattern=[[0, NS], [1, 64]], base=0,
                           channel_multiplier=0)

            # ---- ylow = y & 63 in the [p, (img,k)] layout ----
            ylow = pool.tile([P, J], i32)
            nc.vector.tensor_scalar(out=ylow[:], in0=jnt[:, :, 1], scalar1=63,
                                    scalar2=None, op0=Alu.bitwise_and)

            # ---- gather row index on the folded layout ----
            # rY = 1024*k + 4*x + (y>>6) computed at [p%16, (p//16, img, k)]
            kb16 = pool.tile([16, 2, BPC, K], i32)
            nc.gpsimd.iota(kb16[:], pattern=[[0, 2], [0, BPC], [1024, K]],
                           base=0, channel_multiplier=0)
            rY16 = pool.tile([16, 2, BPC, K], i32)
            nc.vector.tensor_scalar(out=rY16[:], in0=jnt16[:, :, :, :, 1],
                                    scalar1=6, scalar2=None,
                                    op0=Alu.arith_shift_right)
            nc.vector.tensor_tensor(out=rY16[:], in0=rY16[:], in1=kb16[:],
                                    op=Alu.add)
            nc.vector.scalar_tensor_tensor(out=rY16[:],
                                           in0=jnt16[:, :, :, :, 0], scalar=4,
                                           in1=rY16[:], op0=Alu.mult,
                                           op1=Alu.add)
            rYf16 = pool.tile([16, 2 * J], f32)
            nc.vector.tensor_copy(out=rYf16[:],
                                  in_=rY16[:].rearrange("r j b k -> r (j b k)"))

            # replicate [16] -> [128] partitions so each gpsimd core sees the
            # wrapped indices, then one cast into the int16 index tile.
            # g = 32*k + p: idx16 slot [p%16, 2*k + p//16].
            psR = psp.tile([128, 2 * J], f32)
            nc.tensor.matmul(out=psR[:], lhsT=rep[:], rhs=rYf16[:],
                             start=True, stop=True)
            idx16 = pool.tile([128, BPC, NIDX // 16], i16)
            nc.vector.tensor_copy(
                out=idx16[:].rearrange("q b (k j) -> q b j k", j=2),
                in_=psR[:].rearrange("q (j b k) -> q b j k", j=2, b=BPC))

            # ---- the 8 row gathers ----
            rows = pool.tile([128, BPC, NS, 64], f32)
            nc.vector.memset(rows[:, :, NS - 1, :], 0.0)
            for i in range(BPC):
                nc.gpsimd.dma_gather(
                    out_ap=rows[:, i, :, :],
                    in_ap=tags[i],
                    idxs_ap=idx16[:, i, :],
                    num_idxs=NIDX,
                    num_idxs_reg=NIDX,
                    elem_size=64,
                    single_packet=False,
                )

            # ---- scramble ylow / jv-mask into the [32a+p, (i,s)] layout ----
            mjv = pool.tile([P, J], f32)
            nc.vector.tensor_scalar(out=mjv[:], in0=jvt[:], scalar1=0,
                                    scalar2=None, op0=Alu.is_gt)
            ylow_s = pool.tile([128, BPC, NS], i32)
            jvm_s = pool.tile([128, BPC, NS], f32)
            nc.vector.memset(ylow_s[:], 0)
            nc.vector.memset(jvm_s[:], 0.0)
            ylow4 = ylow[:].rearrange("p (b k) -> p b k", k=K)[:, :, 0:16] \
                .rearrange("p b (s a) -> p b a s", a=4)
            mjv4 = mjv[:].rearrange("p (b k) -> p b k", k=K)[:, :, 0:16] \
                .rearrange("p b (s a) -> p b a s", a=4)
            for a in range(4):
                nc.vector.tensor_copy(out=ylow_s[a * 32:a * 32 + P, :, 0:4],
                                      in_=ylow4[:, :, a, :])
                nc.vector.tensor_copy(out=jvm_s[a * 32:a * 32 + P, :, 0:4],
                                      in_=mjv4[:, :, a, :])
            # k = 16 lives in block a=0, slot s=4
            nc.vector.tensor_copy(
                out=ylow_s[0:P, :, 4],
                in_=ylow[:].rearrange("p (b k) -> p b k", k=K)[:, :, 16])
            nc.vector.tensor_copy(
                out=jvm_s[0:P, :, 4],
                in_=mjv[:].rearrange("p (b k) -> p b k", k=K)[:, :, 16])

            # ---- within-window select + masked moments, per image ----
            v_s = pool.tile([128, BPC, NS], f32)
            eq = pool.tile([128, NS, 64], f32)
            prod = pool.tile([128, NS, 64], f32)
            for i in range(BPC):
                nc.vector.tensor_tensor(
                    out=eq[:], in0=iot_w[:],
                    in1=ylow_s[:, i, :].unsqueeze(2).to_broadcast(
                        [128, NS, 64]),
                    op=Alu.is_equal)
                nc.vector.tensor_tensor(out=prod[:], in0=eq[:],
                                        in1=rows[:, i, :, :], op=Alu.mult)
                nc.vector.tensor_reduce(out=v_s[:, i, :], in_=prod[:],
                                        axis=AX.X, op=Alu.add)

            mv = pool.tile([128, BPC, NS], f32)
            mv2 = pool.tile([128, BPC, NS], f32)
            nc.vector.tensor_tensor(out=mv[:], in0=jvm_s[:], in1=v_s[:],
                                    op=Alu.mult)
            nc.vector.tensor_tensor(out=mv2[:], in0=mv[:], in1=v_s[:],
                                    op=Alu.mult)
            stack = pool.tile([128, 3, BPC], f32)
            nc.vector.tensor_reduce(out=stack[:, 0, :], in_=jvm_s[:],
                                    axis=AX.X, op=Alu.add)
            nc.vector.tensor_reduce(out=stack[:, 1, :], in_=mv[:],
                                    axis=AX.X, op=Alu.add)
            nc.vector.tensor_reduce(out=stack[:, 2, :], in_=mv2[:],
                                    axis=AX.X, op=Alu.add)

            # fold 128 -> 30 partitions and apply the person-valid mask
            psM = psp.tile([30, 3, BPC], f32)
            nc.tensor.matmul(out=psM[:].rearrange("p t b -> p (t b)"),
                             lhsT=foldp[:],
                             rhs=stack[:].rearrange("q t b -> q (t b)"),
                             start=True, stop=True)
            mpv = pool.tile([P, BPC], f32)
            nc.vector.tensor_scalar(out=mpv[:], in0=pvt[:], scalar1=0,
                                    scalar2=None, op0=Alu.is_gt)
            mom = pool.tile([P, 3, BPC], f32)
            nc.vector.tensor_tensor(
                out=mom[:], in0=psM[:],
                in1=mpv[:].unsqueeze(1).to_broadcast([P, 3, BPC]),
                op=Alu.mult)
            cnt = mom[:, 0, :]
            s1 = mom[:, 1, :]
            s2 = mom[:, 2, :]

            safe = pool.tile([P, BPC], f32)
            inv = pool.tile([P, BPC], f32)
            nc.vector.tensor_scalar(out=safe[:], in0=cnt, scalar1=1.0,
                                    scalar2=None, op0=Alu.max)
            nc.vector.reciprocal(out=inv[:], in_=safe[:])

            # staging tile for the 32x32 block transpose:
            # block0 = mean, block1 = pm, block2 = pull_p
            t_in = pool.tile([32, 96], f32)
            t_out = pool.tile([32, 96], f32)
            nc.vector.memset(t_in[:], 0.0)
            mean = t_in[0:P, 0:BPC]
            pm = t_in[0:P, 32:32 + BPC]
            pullp = t_in[0:P, 64:64 + BPC]
            nc.vector.tensor_tensor(out=mean, in0=s1, in1=inv[:], op=Alu.mult)
            nc.vector.tensor_scalar(out=pm, in0=cnt, scalar1=0.0,
                                    scalar2=None, op0=Alu.is_gt)
            tmp = pool.tile([P, BPC], f32)
            nc.vector.tensor_tensor(out=tmp[:], in0=s1, in1=mean, op=Alu.mult)
            nc.vector.tensor_tensor(out=tmp[:], in0=s2, in1=tmp[:],
                                    op=Alu.subtract)
            nc.vector.tensor_tensor(out=pullp, in0=tmp[:], in1=inv[:],
                                    op=Alu.mult)

            nc.vector.transpose(out=t_out[:], in_=t_in[:])
            meanT = t_out[0:BPC, 0:P]          # [8, 30]
            pmT = t_out[0:BPC, 32:32 + P]
            pullT = t_out[0:BPC, 64:64 + P]

            # ---- push: pairwise exp(-(mean_p - mean_q)^2) ----
            d2 = pool.tile([BPC, P, P], f32)
            nc.vector.tensor_tensor(
                out=d2[:],
                in0=meanT.unsqueeze(2).to_broadcast([BPC, P, P]),
                in1=meanT.unsqueeze(1).to_broadcast([BPC, P, P]),
                op=Alu.subtract)
            nc.vector.tensor_tensor(out=d2[:], in0=d2[:], in1=d2[:],
                                    op=Alu.mult)
            e = pool.tile([BPC, P, P], f32)
            nc.scalar.activation(out=e[:], in_=d2[:], func=Act.Exp, scale=-1.0)
            pair = pool.tile([BPC, P, P], f32)
            nc.vector.tensor_tensor(
                out=pair[:],
                in0=pmT.unsqueeze(2).to_broadcast([BPC, P, P]),
                in1=pmT.unsqueeze(1).to_broadcast([BPC, P, P]),
                op=Alu.mult)
            ep = pool.tile([BPC, P, P], f32)
            s_img = pool.tile([BPC, 1], f32)
            nc.vector.tensor_tensor(out=ep[:], in0=e[:], in1=pair[:],
                                    op=Alu.mult)
            nc.vector.tensor_reduce(out=s_img[:], in_=ep[:], axis=AX.XY,
                                    op=Alu.add)

            n_img = pool.tile([BPC, 1], f32)
            pull_sum = pool.tile([BPC, 1], f32)
            nc.vector.tensor_reduce(out=n_img[:], in_=pmT, axis=AX.X,
                                    op=Alu.add)
            nc.vector.tensor_reduce(out=pull_sum[:], in_=pullT, axis=AX.X,
                                    op=Alu.add)

            # push_img = (s - n) * 0.5 / max(n^2-n, 1) * (n > 1)
            pp2 = pool.tile([BPC, 2], f32)
            den = pool.tile([BPC, 1], f32)
            dinv = pool.tile([BPC, 1], f32)
            g = pool.tile([BPC, 1], f32)
            u = pool.tile([BPC, 1], f32)
            nc.vector.tensor_tensor(out=den[:], in0=n_img[:], in1=n_img[:],
                                    op=Alu.mult)
            nc.vector.tensor_tensor(out=den[:], in0=den[:], in1=n_img[:],
                                    op=Alu.subtract)
            nc.vector.tensor_scalar(out=den[:], in0=den[:], scalar1=1.0,
                                    scalar2=None, op0=Alu.max)
            nc.vector.reciprocal(out=dinv[:], in_=den[:])
            nc.vector.tensor_scalar(out=g[:], in0=n_img[:], scalar1=1.0,
                                    scalar2=None, op0=Alu.is_gt)
            nc.vector.tensor_tensor(out=u[:], in0=s_img[:], in1=n_img[:],
                                    op=Alu.subtract)
            nc.vector.tensor_tensor(out=u[:], in0=u[:], in1=dinv[:],
                                    op=Alu.mult)
            nc.vector.tensor_tensor(out=u[:], in0=u[:], in1=g[:], op=Alu.mult)
            nc.vector.tensor_scalar(out=pp2[:, 0:1], in0=u[:], scalar1=0.5,
                                    scalar2=None, op0=Alu.mult)

            # pull_img = pull_sum / max(n, 1)
            nm = pool.tile([BPC, 1], f32)
            ninv = pool.tile([BPC, 1], f32)
            nc.vector.tensor_scalar(out=nm[:], in0=n_img[:], scalar1=1.0,
                                    scalar2=None, op0=Alu.max)
            nc.vector.reciprocal(out=ninv[:], in_=nm[:])
            nc.vector.tensor_tensor(out=pp2[:, 1:2], in0=pull_sum[:],
                                    in1=ninv[:], op=Alu.mult)

            # ---- sum over the 8 local images: psum[2,1] = pp2^T @ ones ----
            ones8 = pool.tile([BPC, 1], f32)
            nc.vector.memset(ones8[:], 1.0)
            acc = psp.tile([2, 1], f32)
            nc.tensor.matmul(out=acc[:], lhsT=pp2[:], rhs=ones8[:],
                             start=True, stop=True)
            res = pool.tile([2, 1], f32)
            nc.scalar.activation(out=res[:], in_=acc[:], func=Act.Copy,
                                 scale=1.0 / B)

            if collective:
                with tc.tile_pool(name="dram", bufs=1, space="DRAM") as dram:
                    ar_in = dram.tile([2, 1], f32)
                    ar_out = dram.tile([2, 1], f32)
                    nc.sync.dma_start(out=ar_in[:], in_=res[:])
                    nc.gpsimd.collective_compute(
                        "AllReduce", Alu.add,
                        replica_groups=[list(range(NCORES))],
                        ins=[ar_in.opt()],
                        outs=[ar_out.opt()],
                    )
                    nc.sync.dma_start(out=out[:], in_=ar_out[:])
            else:
                nc.sync.dma_start(out=out[:], in_=res[:])

    nc.compile()
    return nc


_nc_cache = {}


def _get_nc(collective: bool):
    if collective not in _nc_cache:
        _nc_cache[collective] = build_nc(collective)
    return _nc_cache[collective]


def _shard_tags(shard):
    return np.ascontiguousarray(shard.reshape(BPC, NROW, 64))


def kernel(tags, joints, joint_img_valid, person_valid):
    collective = os.environ.get("AELOSS_COLLECTIVE", "0") == "1"
    nc = _get_nc(collective)

    tags = np.asarray(tags, dtype=np.float32)
    joints = np.ascontiguousarray(np.asarray(joints, dtype=np.int32))
    jv = np.ascontiguousarray(np.asarray(joint_img_valid, dtype=np.int32))
    pv = np.ascontiguousarray(np.asarray(person_valid, dtype=np.int32))

    in_maps = []
    for c in range(NCORES):
        sl = slice(c * BPC, (c + 1) * BPC)
        in_maps.append({
            "tags": _shard_tags(tags[sl]),
            "joints": joints[sl],
            "jv": jv[sl],
            "pv": pv[sl],
        })

    res = bass_utils.run_bass_kernel_spmd(nc, in_maps,
                                          core_ids=list(range(NCORES)))
    outs = [np.asarray(r["out"], dtype=np.float64).reshape(2)
            for r in res.results]
    if collective:
        total = outs[0]
    else:
        total = np.sum(outs, axis=0)
    return np.float32(total[0]), np.float32(total[1])


if __name__ == "__main__":
    rng = np.random.default_rng(0)
    t = rng.standard_normal((B, K, H, W), dtype=np.float32)
    j = rng.integers(0, H, size=(B, P, K, 2), dtype=np.int32)
    jv_ = rng.integers(0, 2, size=(B, P, K), dtype=np.int32)
    pv_ = rng.integers(0, 2, size=(B, P), dtype=np.int32)
    print(kernel(t, j, jv_, pv_))


# revision 18
# speedup vs baseline: 1.2315x; 1.0325x over previous
"""Associative-embedding (push/pull) loss on 8 TRN2 NeuronCores.

Data-parallel: 8 images per core. The 285MB tags tensor is only touched at
P*K=510 points per image, so each core row-gathers 64-element windows with
dma_gather (one CounterMachine-accelerated SWDGE instruction per image,
640 int16 row indices each) instead of streaming the tensor:

  element e = 65536*(img*17+k) + 256*x + y
  row r (64-wide) = k*1024 + 4*x + (y>>6), selected within-window by y&63.

Gathered rows land as out[g%128, g//128, :] with g = 128*(k//4) + 32*(k%4)
+ p, i.e. partition q = 32*(k%4)+p -> a 120-partition-parallel layout for
the one-hot within-window select and the masked moment sums. A 128->30
fold matmul brings per-(person,img) moments back to person partitions;
pairwise push runs in an [img, person] layout after a 32x32 DVE block
transpose. Per-core (sum_push, sum_pull)/64 partials are summed across
cores (host side, or on-device AllReduce with AELOSS_COLLECTIVE=1).
"""

import os
import sys

import numpy as np

if "/opt/trn_rl_repo" not in sys.path:
    sys.path.insert(0, "/opt/trn_rl_repo")

from concourse import bacc, bass, mybir, tile  # noqa: E402
from concourse import bass_utils  # noqa: E402
from concourse.masks import make_identity  # noqa: E402

B, P, K, H, W = 64, 30, 17, 256, 256
NCORES = 8
BPC = B // NCORES           # 8 images per core
J = BPC * K                 # 136 (img, k) columns
KHW = K * H * W             # 1114112
NTOT = BPC * KHW
NROW = KHW // 64            # 17408 64-elem rows per image
NIDX = 544                  # 17 k * 32 partitions per image (g = 32k + p)
NS = 5                      # k slots per image (slot = k // 4)

f32 = mybir.dt.float32
i32 = mybir.dt.int32
i16 = mybir.dt.int16
Alu = mybir.AluOpType
Act = mybir.ActivationFunctionType
AX = mybir.AxisListType


def build_nc(collective: bool = False):
    nc = bacc.Bacc("TRN2", target_bir_lowering=False, debug=False,
                   num_devices=NCORES)

    tags = nc.dram_tensor("tags", [BPC, NROW, 64], f32, kind="ExternalInput")
    joints = nc.dram_tensor("joints", [BPC, P, K, 2], i32, kind="ExternalInput")
    jv = nc.dram_tensor("jv", [BPC, P, K], i32, kind="ExternalInput")
    pv = nc.dram_tensor("pv", [BPC, P], i32, kind="ExternalInput")
    out = nc.dram_tensor("out", [2, 1], f32, kind="ExternalOutput")

    with tile.TileContext(nc) as tc:
        with tc.tile_pool(name="sbuf", bufs=1) as pool, \
             tc.tile_pool(name="psum", bufs=1, space="PSUM") as psp:

            # ===== critical chain first (trace order drives scheduling) =====
            # folded joints [p%16, p//16, img, k, xy] -> gather indices.
            jnt16 = pool.tile([16, 2, BPC, K, 2], i32)
            nc.sync.dma_start(
                out=jnt16[:, 0],
                in_=joints[:, 0:16].rearrange("b r k c -> r b k c"))
            nc.sync.dma_start(
                out=jnt16[0:14, 1],
                in_=joints[:, 16:30].rearrange("b r k c -> r b k c"))

            # replicate matrix [16, 128]: rep[r, q] = (q % 16 == r)
            iot_f16 = pool.tile([16, 128], i32)
            iot_c16 = pool.tile([16, 128], i32)
            rep = pool.tile([16, 128], f32)
            nc.gpsimd.iota(iot_f16[:], pattern=[[0, 8], [1, 16]], base=0,
                           channel_multiplier=0)
            nc.gpsimd.iota(iot_c16[:], pattern=[[0, 128]], base=0,
                           channel_multiplier=1)
            nc.vector.tensor_tensor(out=rep[:], in0=iot_f16[:], in1=iot_c16[:],
                                    op=Alu.is_equal)
            kb16 = pool.tile([16, 2, BPC, K], i32)
            nc.gpsimd.iota(kb16[:], pattern=[[0, 2], [0, BPC], [1024, K]],
                           base=0, channel_multiplier=0)

            # rY = 1024*k + 4*x + (y>>6), clamped to valid rows (partitions
            # 14..15 of the j2=1 half hold garbage for persons 30..31).
            rY16 = pool.tile([16, 2, BPC, K], i32)
            nc.vector.tensor_scalar(out=rY16[:], in0=jnt16[:, :, :, :, 1],
                                    scalar1=6, scalar2=None,
                                    op0=Alu.arith_shift_right)
            nc.vector.tensor_tensor(out=rY16[:], in0=rY16[:], in1=kb16[:],
                                    op=Alu.add)
            nc.vector.scalar_tensor_tensor(out=rY16[:],
                                           in0=jnt16[:, :, :, :, 0], scalar=4,
                                           in1=rY16[:], op0=Alu.mult,
                                           op1=Alu.add)
            nc.vector.tensor_scalar(out=rY16[:], in0=rY16[:], scalar1=0,
                                    scalar2=NROW - 1, op0=Alu.max, op1=Alu.min)
            rYf16 = pool.tile([16, 2 * J], f32)
            nc.vector.tensor_copy(out=rYf16[:],
                                  in_=rY16[:].rearrange("r j b k -> r (j b k)"))

            # replicate [16] -> [128] partitions so each gpsimd core sees the
            # wrapped indices, then one cast into the int16 index tile.
            # g = 32*k + p: idx16 slot [p%16, 2*k + p//16].
            psR = psp.tile([128, 2 * J], f32)
            nc.tensor.matmul(out=psR[:], lhsT=rep[:], rhs=rYf16[:],
                             start=True, stop=True)
            idx16 = pool.tile([128, BPC, NIDX // 16], i16)
            nc.vector.tensor_copy(
                out=idx16[:].rearrange("q b (k j) -> q b j k", j=2),
                in_=psR[:].rearrange("q (j b k) -> q b j k", j=2, b=BPC))

            # ---- the 8 row gathers ----
            rows = pool.tile([128, BPC, NS, 64], f32)
            nc.vector.memset(rows[:, :, NS - 1, :], 0.0)
            for i in range(BPC):
                nc.gpsimd.dma_gather(
                    out_ap=rows[:, i, :, :],
                    in_ap=tags[i],
                    idxs_ap=idx16[:, i, :],
                    num_idxs=NIDX,
                    num_idxs_reg=NIDX,
                    elem_size=64,
                    single_packet=False,
                )

            # ===== off-critical-path work (overlaps the gathers) =====
            # [p, (img,k)] joints / masks on the scalar-engine DMA queue.
            jnt = pool.tile([P, J, 2], i32)
            jvt = pool.tile([P, J], i32)
            pvt = pool.tile([P, BPC], i32)
            nc.scalar.dma_start(out=jnt[:],
                                in_=joints[:].rearrange("b p k c -> p b k c"))
            nc.scalar.dma_start(out=jvt[:],
                                in_=jv[:].rearrange("b p k -> p b k"))
            nc.scalar.dma_start(out=pvt[:], in_=pv[:].rearrange("b p -> p b"))

            # fold matrix [128, 30]: foldp[q, p] = (q % 32 == p)
            iot_q = pool.tile([128, 1], i32)
            iot_r30 = pool.tile([128, 30], i32)
            foldp = pool.tile([128, 30], f32)
            nc.gpsimd.iota(iot_q[:], pattern=[[0, 1]], base=0,
                           channel_multiplier=1)
            nc.vector.tensor_scalar(out=iot_q[:], in0=iot_q[:], scalar1=31,
                                    scalar2=None, op0=Alu.bitwise_and)
            nc.gpsimd.iota(iot_r30[:], pattern=[[1, 30]], base=0,
                           channel_multiplier=0)
            nc.vector.tensor_tensor(out=foldp[:], in0=iot_r30[:],
                                    in1=iot_q[:].to_broadcast([128, 30]),
                                    op=Alu.is_equal)
            # within-window iota [128, NS, 64] (value = w)
            iot_w = pool.tile([128, NS, 64], i32)
            nc.gpsimd.iota(iot_w[:], pattern=[[0, NS], [1, 64]], base=0,
                           channel_multiplier=0)

            # ---- ylow = y & 63 in the [p, (img,k)] layout ----
            ylow = pool.tile([P, J], i32)
            nc.vector.tensor_scalar(out=ylow[:], in0=jnt[:, :, 1], scalar1=63,
                                    scalar2=None, op0=Alu.bitwise_and)

            # ---- scramble ylow / jv-mask into the [32a+p, (i,s)] layout ----
            mjv = pool.tile([P, J], f32)
            nc.vector.tensor_scalar(out=mjv[:], in0=jvt[:], scalar1=0,
                                    scalar2=None, op0=Alu.is_gt)
            ylow_s = pool.tile([128, BPC, NS], i32)
            jvm_s = pool.tile([128, BPC, NS], f32)
            nc.vector.memset(ylow_s[:], 0)
            nc.vector.memset(jvm_s[:], 0.0)
            ylow4 = ylow[:].rearrange("p (b k) -> p b k", k=K)[:, :, 0:16] \
                .rearrange("p b (s a) -> p b a s", a=4)
            mjv4 = mjv[:].rearrange("p (b k) -> p b k", k=K)[:, :, 0:16] \
                .rearrange("p b (s a) -> p b a s", a=4)
            for a in range(4):
                nc.vector.tensor_copy(out=ylow_s[a * 32:a * 32 + P, :, 0:4],
                                      in_=ylow4[:, :, a, :])
                nc.vector.tensor_copy(out=jvm_s[a * 32:a * 32 + P, :, 0:4],
                                      in_=mjv4[:, :, a, :])
            # k = 16 lives in block a=0, slot s=4
            nc.vector.tensor_copy(
                out=ylow_s[0:P, :, 4],
                in_=ylow[:].rearrange("p (b k) -> p b k", k=K)[:, :, 16])
            nc.vector.tensor_copy(
                out=jvm_s[0:P, :, 4],
                in_=mjv[:].rearrange("p (b k) -> p b k", k=K)[:, :, 16])

            # ---- within-window select + masked moments, per image ----
            v_s = pool.tile([128, BPC, NS], f32)
            eq = pool.tile([128, NS, 64], f32)
            prod = pool.tile([128, NS, 64], f32)
            for i in range(BPC):
                nc.vector.tensor_tensor(
                    out=eq[:], in0=iot_w[:],
                    in1=ylow_s[:, i, :].unsqueeze(2).to_broadcast(
                        [128, NS, 64]),
                    op=Alu.is_equal)
                nc.vector.tensor_tensor(out=prod[:], in0=eq[:],
                                        in1=rows[:, i, :, :], op=Alu.mult)
                nc.vector.tensor_reduce(out=v_s[:, i, :], in_=prod[:],
                                        axis=AX.X, op=Alu.add)

            mv = pool.tile([128, BPC, NS], f32)
            mv2 = pool.tile([128, BPC, NS], f32)
            nc.vector.tensor_tensor(out=mv[:], in0=jvm_s[:], in1=v_s[:],
                                    op=Alu.mult)
            nc.vector.tensor_tensor(out=mv2[:], in0=mv[:], in1=v_s[:],
                                    op=Alu.mult)
            stack = pool.tile([128, 3, BPC], f32)
            nc.vector.tensor_reduce(out=stack[:, 0, :], in_=jvm_s[:],
                                    axis=AX.X, op=Alu.add)
            nc.vector.tensor_reduce(out=stack[:, 1, :], in_=mv[:],
                                    axis=AX.X, op=Alu.add)
            nc.vector.tensor_reduce(out=stack[:, 2, :], in_=mv2[:],
                                    axis=AX.X, op=Alu.add)

            # fold 128 -> 30 partitions and apply the person-valid mask
            psM = psp.tile([30, 3, BPC], f32)
            nc.tensor.matmul(out=psM[:].rearrange("p t b -> p (t b)"),
                             lhsT=foldp[:],
                             rhs=stack[:].rearrange("q t b -> q (t b)"),
                             start=True, stop=True)
            mpv = pool.tile([P, BPC], f32)
            nc.vector.tensor_scalar(out=mpv[:], in0=pvt[:], scalar1=0,
                                    scalar2=None, op0=Alu.is_gt)
            mom = pool.tile([P, 3, BPC], f32)
            nc.vector.tensor_tensor(
                out=mom[:], in0=psM[:],
                in1=mpv[:].unsqueeze(1).to_broadcast([P, 3, BPC]),
                op=Alu.mult)
            cnt = mom[:, 0, :]
            s1 = mom[:, 1, :]
            s2 = mom[:, 2, :]

            safe = pool.tile([P, BPC], f32)
            inv = pool.tile([P, BPC], f32)
            nc.vector.tensor_scalar(out=safe[:], in0=cnt, scalar1=1.0,
                                    scalar2=None, op0=Alu.max)
            nc.vector.reciprocal(out=inv[:], in_=safe[:])

            # staging tile for the 32x32 block transpose:
            # block0 = mean, block1 = pm, block2 = pull_p
            t_in = pool.tile([32, 96], f32)
            t_out = pool.tile([32, 96], f32)
            nc.vector.memset(t_in[:], 0.0)
            mean = t_in[0:P, 0:BPC]
            pm = t_in[0:P, 32:32 + BPC]
            pullp = t_in[0:P, 64:64 + BPC]
            nc.vector.tensor_tensor(out=mean, in0=s1, in1=inv[:], op=Alu.mult)
            nc.vector.tensor_scalar(out=pm, in0=cnt, scalar1=0.0,
                                    scalar2=None, op0=Alu.is_gt)
            tmp = pool.tile([P, BPC], f32)
            nc.vector.tensor_tensor(out=tmp[:], in0=s1, in1=mean, op=Alu.mult)
            nc.vector.tensor_tensor(out=tmp[:], in0=s2, in1=tmp[:],
                                    op=Alu.subtract)
            nc.vector.tensor_tensor(out=pullp, in0=tmp[:], in1=inv[:],
                                    op=Alu.mult)

            nc.vector.transpose(out=t_out[:], in_=t_in[:])
            meanT = t_out[0:BPC, 0:P]          # [8, 30]
            pmT = t_out[0:BPC, 32:32 + P]
            pullT = t_out[0:BPC, 64:64 + P]

            # ---- push: pairwise exp(-(mean_p - mean_q)^2) ----
            d2 = pool.tile([BPC, P, P], f32)
            nc.vector.tensor_tensor(
                out=d2[:],
                in0=meanT.unsqueeze(2).to_broadcast([BPC, P, P]),
                in1=meanT.unsqueeze(1).to_broadcast([BPC, P, P]),
                op=Alu.subtract)
            nc.vector.tensor_tensor(out=d2[:], in0=d2[:], in1=d2[:],
                                    op=Alu.mult)
            e = pool.tile([BPC, P, P], f32)
            nc.scalar.activation(out=e[:], in_=d2[:], func=Act.Exp, scale=-1.0)
            pair = pool.tile([BPC, P, P], f32)
            nc.vector.tensor_tensor(
                out=pair[:],
                in0=pmT.unsqueeze(2).to_broadcast([BPC, P, P]),
                in1=pmT.unsqueeze(1).to_broadcast([BPC, P, P]),
                op=Alu.mult)
            ep = pool.tile([BPC, P, P], f32)
            s_img = pool.tile([BPC, 1], f32)
            nc.vector.tensor_tensor(out=ep[:], in0=e[:], in1=pair[:],
                                    op=Alu.mult)
            nc.vector.tensor_reduce(out=s_img[:], in_=ep[:], axis=AX.XY,
                                    op=Alu.add)

            n_img = pool.tile([BPC, 1], f32)
            pull_sum = pool.tile([BPC, 1], f32)
            nc.vector.tensor_reduce(out=n_img[:], in_=pmT, axis=AX.X,
                                    op=Alu.add)
            nc.vector.tensor_reduce(out=pull_sum[:], in_=pullT, axis=AX.X,
                                    op=Alu.add)

            # push_img = (s - n) * 0.5 / max(n^2-n, 1) * (n > 1)
            pp2 = pool.tile([BPC, 2], f32)
            den = pool.tile([BPC, 1], f32)
            dinv = pool.tile([BPC, 1], f32)
            g = pool.tile([BPC, 1], f32)
            u = pool.tile([BPC, 1], f32)
            nc.vector.tensor_tensor(out=den[:], in0=n_img[:], in1=n_img[:],
                                    op=Alu.mult)
            nc.vector.tensor_tensor(out=den[:], in0=den[:], in1=n_img[:],
                                    op=Alu.subtract)
            nc.vector.tensor_scalar(out=den[:], in0=den[:], scalar1=1.0,
                                    scalar2=None, op0=Alu.max)
            nc.vector.reciprocal(out=dinv[:], in_=den[:])
            nc.vector.tensor_scalar(out=g[:], in0=n_img[:], scalar1=1.0,
                                    scalar2=None, op0=Alu.is_gt)
            nc.vector.tensor_tensor(out=u[:], in0=s_img[:], in1=n_img[:],
                                    op=Alu.subtract)
            nc.vector.tensor_tensor(out=u[:], in0=u[:], in1=dinv[:],
                                    op=Alu.mult)
            nc.vector.tensor_tensor(out=u[:], in0=u[:], in1=g[:], op=Alu.mult)
            nc.vector.tensor_scalar(out=pp2[:, 0:1], in0=u[:], scalar1=0.5,
                                    scalar2=None, op0=Alu.mult)

            # pull_img = pull_sum / max(n, 1)
            nm = pool.tile([BPC, 1], f32)
            ninv = pool.tile([BPC, 1], f32)
            nc.vector.tensor_scalar(out=nm[:], in0=n_img[:], scalar1=1.0,
                                    scalar2=None, op0=Alu.max)
            nc.vector.reciprocal(out=ninv[:], in_=nm[:])
            nc.vector.tensor_tensor(out=pp2[:, 1:2], in0=pull_sum[:],
                                    in1=ninv[:], op=Alu.mult)

            # ---- sum over the 8 local images: psum[2,1] = pp2^T @ ones ----
            ones8 = pool.tile([BPC, 1], f32)
            nc.vector.memset(ones8[:], 1.0)
            acc = psp.tile([2, 1], f32)
            nc.tensor.matmul(out=acc[:], lhsT=pp2[:], rhs=ones8[:],
                             start=True, stop=True)
            res = pool.tile([2, 1], f32)
            nc.scalar.activation(out=res[:], in_=acc[:], func=Act.Copy,
                                 scale=1.0 / B)

            if collective:
                with tc.tile_pool(name="dram", bufs=1, space="DRAM") as dram:
                    ar_in = dram.tile([2, 1], f32)
                    ar_out = dram.tile([2, 1], f32)
                    nc.sync.dma_start(out=ar_in[:], in_=res[:])
                    nc.gpsimd.collective_compute(
                        "AllReduce", Alu.add,
                        replica_groups=[list(range(NCORES))],
                        ins=[ar_in.opt()],
                        outs=[ar_out.opt()],
                    )
                    nc.sync.dma_start(out=out[:], in_=ar_out[:])
            else:
                nc.sync.dma_start(out=out[:], in_=res[:])

    nc.compile()
    return nc


_nc_cache = {}


def _get_nc(collective: bool):
    if collective not in _nc_cache:
        _nc_cache[collective] = build_nc(collective)
    return _nc_cache[collective]


def _shard_tags(shard):
    return np.ascontiguousarray(shard.reshape(BPC, NROW, 64))


def kernel(tags, joints, joint_img_valid, person_valid):
    collective = os.environ.get("AELOSS_COLLECTIVE", "0") == "1"
    nc = _get_nc(collective)

    tags = np.asarray(tags, dtype=np.float32)
    joints = np.ascontiguousarray(np.asarray(joints, dtype=np.int32))
    jv = np.ascontiguousarray(np.asarray(joint_img_valid, dtype=np.int32))
    pv = np.ascontiguousarray(np.asarray(person_valid, dtype=np.int32))

    in_maps = []
    for c in range(NCORES):
        sl = slice(c * BPC, (c + 1) * BPC)
        in_maps.append({
            "tags": _shard_tags(tags[sl]),
            "joints": joints[sl],
            "jv": jv[sl],
            "pv": pv[sl],
        })

    res = bass_utils.run_bass_kernel_spmd(nc, in_maps,
                                          core_ids=list(range(NCORES)))
    outs = [np.asarray(r["out"], dtype=np.float64).reshape(2)
            for r in res.results]
    if collective:
        total = outs[0]
    else:
        total = np.sum(outs, axis=0)
    return np.float32(total[0]), np.float32(total[1])


if __name__ == "__main__":
    rng = np.random.default_rng(0)
    t = rng.standard_normal((B, K, H, W), dtype=np.float32)
    j = rng.integers(0, H, size=(B, P, K, 2), dtype=np.int32)
    jv_ = rng.integers(0, 2, size=(B, P, K), dtype=np.int32)
    pv_ = rng.integers(0, 2, size=(B, P), dtype=np.int32)
    print(kernel(t, j, jv_, pv_))
